# revision 29
# baseline (speedup 1.0000x reference)
"""AffinityFC Trainium2 kernel (Bass/Tile, 8 NeuronCores, data-parallel over B).

Math per batch b (one NeuronCore per batch):
    px = X[b] @ W1x.T          (Nx=128, hd=1024)
    py = Y[b] @ W1y.T          (Ny=128, hd=1024)
    out[n, m] = W2 . relu(px[n, :] + py[m, :] + b1) + b2

Key reformulation: with s = px + b1,
    relu(py + s) = max(py, -s) + s
so the device computes u = max(py, -s) (a plain tensor_tensor max, no
broadcast materialization, no relu pass) and reduces Sum_h W2[h]*u with
TensorE; the Sum_h W2[h]*s[n,h] term is a rank-1 correction
gamma[b,n] = (W2@W1x)·X[b,n] + W2·b1 added on the host, along with b2.

Device layout (per core):
  - layer 1 (TensorE, bf16): per h-chunk c (8 chunks of 128):
      negs_c = -(px_c + b1_c)         (h, n) bf16   [DVE psum evac]
      py_rep4_c[hp, m*4+j] = py_c[hp, m]  (x4 interleaved) bf16 [ACT evac]
  - main loop, c-outer: for each chunk c, 4 octet TT ops produce
      t[hp, nbs*512 + m*4 + j] = max(py_c[hp,m], negs_c[hp, 4*nb+j])
    at 2x bf16 (all operands innermost step-1), then 32 matmuls with the
    same stationary W2_c column accumulate psum slivers
    (bank=nbs, partition=32*oct).  Sliver evacuation on ScalarE.
  - output raw layout: raw[nb*512 + m*4 + j] = out[4nb+j, m]; host
    unscrambles and adds gamma + b2.
"""

import numpy as np
import ml_dtypes

import concourse.mybir as mybir
import concourse.tile as tile
from concourse import bacc
from concourse.bass import ts
from concourse.bass_utils import run_bass_kernel_spmd

B, NX, NY, D, HD = 8, 128, 128, 512, 1024
NCORES = 8
NCH = HD // 128      # 8 h-chunks
KT = D // 128        # 4 k-tiles for the layer-1 contraction
NBLK = NX // 4       # 32 n-blocks of 4 rows each
OCTW = 8             # n-blocks per TT producer op (FD = OCTW*512)
NOCT = NBLK // OCTW  # 4 octets
F32 = mybir.dt.float32
BF16 = mybir.dt.bfloat16

# chunks of each octet handed to GPSIMD (scalar_tensor_tensor) instead of DVE
# (walrus rejects >3D STT APs, so this stays empty for the 4D octet op)
POOL_CHUNKS = ()


def _build_nc(do_compile=True):
    nc = bacc.Bacc(
        "TRN2", target_bir_lowering=False, debug=False, num_devices=NCORES
    )

    xt = nc.dram_tensor("xt", [D, NX], BF16, kind="ExternalInput")
    yt = nc.dram_tensor("yt", [D, NY], BF16, kind="ExternalInput")
    # chunk-major: rows c*D..(c+1)*D hold W1{x,y}.T[:, c*128:(c+1)*128]
    w1xt = nc.dram_tensor("w1xt", [NCH * D, 128], BF16, kind="ExternalInput")
    w1yt = nc.dram_tensor("w1yt", [NCH * D, 128], BF16, kind="ExternalInput")
    b1c = nc.dram_tensor("b1c", [128, NCH], F32, kind="ExternalInput")
    w2c = nc.dram_tensor("w2c", [128, NCH], BF16, kind="ExternalInput")
    out = nc.dram_tensor("out", [1, NBLK * 512], F32, kind="ExternalOutput")

    with tile.TileContext(nc) as tc:
        with (
            tc.tile_pool(name="const", bufs=1) as cp,
            tc.tile_pool(name="tprod", bufs=5) as tp,
        ):
            xt_sb = cp.tile([128, KT * NX], BF16)
            yt_sb = cp.tile([128, KT * NY], BF16)
            w1x_sb = cp.tile([128, KT * HD], BF16)
            w1y_sb = cp.tile([128, KT * HD], BF16)
            b1_sb = cp.tile([128, NCH], F32)
            w2_sb = cp.tile([128, NCH], BF16)
            negs_sb = cp.tile([128, HD], BF16)
            pyr_sb = cp.tile([128, NCH * 512], BF16)  # py_rep4 per chunk
            # separate staging tiles per evac engine (banks 0-3 -> ACT,
            # banks 4-7 -> DVE) so the two evac chains run in parallel
            out_sbA = cp.tile([1, NBLK * 256], F32)
            out_sbB = cp.tile([1, NBLK * 256], F32)

            nc.sync.dma_start(out=b1_sb[:, :], in_=b1c[:, :])
            nc.sync.dma_start(out=w2_sb[:, :], in_=w2c[:, :])
            nc.sync.dma_start(
                out=xt_sb[:, :].rearrange("p (k n) -> p k n", k=KT),
                in_=xt[:, :].rearrange("(k p) n -> p k n", p=128),
            )
            nc.sync.dma_start(
                out=yt_sb[:, :].rearrange("p (k n) -> p k n", k=KT),
                in_=yt[:, :].rearrange("(k p) n -> p k n", p=128),
            )
            # W1 in half-chunks spread over the three DGE queues; chunk-major
            # source keeps per-partition segments contiguous.
            half = NCH * D // 2  # rows per half (chunks 0-3 / 4-7)
            for w_sb, w_dr, engs in (
                (w1x_sb, w1xt, (nc.scalar, nc.scalar)),
                (w1y_sb, w1yt, (nc.gpsimd, nc.gpsimd)),
            ):
                for hf in range(2):
                    engs[hf].dma_start(
                        out=w_sb[
                            :, hf * (KT * NCH // 2) * 128 : (hf + 1)
                            * (KT * NCH // 2)
                            * 128
                        ].rearrange("p (ck h) -> p ck h", h=128),
                        in_=w_dr[hf * half : (hf + 1) * half, :].rearrange(
                            "(ck p) h -> p ck h", p=128
                        ),
                    )

            # ---- layer 1 per h-chunk: negs (DVE) + py_rep4 (ACT)
            with tc.tile_pool(name="l1ps", bufs=4, space="PSUM") as l1ps:
                for c in range(NCH):
                    pxp = l1ps.tile([128, NX], F32, tag="l1")
                    for k in range(KT):
                        nc.tensor.matmul(
                            pxp[:, :],
                            w1x_sb[:, (c * KT + k) * 128 : (c * KT + k + 1) * 128],
                            xt_sb[:, ts(k, NX)],
                            start=(k == 0),
                            stop=(k == KT - 1),
                        )
                    nc.vector.tensor_scalar(
                        out=negs_sb[:, ts(c, 128)],
                        in0=pxp[:, :],
                        scalar1=b1_sb[:, c : c + 1],
                        scalar2=-1.0,
                        op0=mybir.AluOpType.add,
                        op1=mybir.AluOpType.mult,
                    )
                    pyp = l1ps.tile([128, NY], F32, tag="l1")
                    for k in range(KT):
                        nc.tensor.matmul(
                            pyp[:, :],
                            w1y_sb[:, (c * KT + k) * 128 : (c * KT + k + 1) * 128],
                            yt_sb[:, ts(k, NY)],
                            start=(k == 0),
                            stop=(k == KT - 1),
                        )
                    nc.scalar.activation(
                        out=pyr_sb[:, ts(c, 512)].rearrange(
                            "p (m j) -> p m j", j=4
                        ),
                        in_=pyp[:, :].unsqueeze(2).broadcast_to((128, 128, 4)),
                        func=mybir.ActivationFunctionType.Copy,
                    )

            # ---- main loop, c-outer: all 32 psum slivers stay resident and
            # accumulate across the 8 chunk passes; W2_c stays stationary
            # within a pass.  The last pass runs bank-major with evacuations
            # interleaved (a bank's 4 slivers evac while later banks matmul).
            with tc.tile_pool(name="mps", bufs=1, space="PSUM") as mps:
                obanks = [
                    mps.tile([128, 512], F32, name=f"ob{i}", tag=f"ob{i}")
                    for i in range(8)
                ]
                for c in range(NCH):
                    last = c == NCH - 1
                    pyr_c = pyr_sb[:, ts(c, 512)]
                    tts = []
                    for oct_ in range(NOCT):
                        t = tp.tile(
                            [128, OCTW * 512], BF16, name=f"t{c}_{oct_}", tag="t"
                        )
                        in0 = (
                            pyr_c.rearrange("p (m j) -> p m j", j=4)
                            .unsqueeze(1)
                            .broadcast_to((128, OCTW, 128, 4))
                        )
                        base = c * 128 + oct_ * (4 * OCTW)
                        in1 = (
                            negs_sb[:, base : base + 4 * OCTW]
                            .rearrange("p (nbs j) -> p nbs j", j=4)
                            .unsqueeze(2)
                            .broadcast_to((128, OCTW, 128, 4))
                        )
                        nc.vector.tensor_tensor(
                            out=t[:, :].rearrange(
                                "p (nbs m j) -> p nbs m j", nbs=OCTW, m=128
                            ),
                            in0=in0,
                            in1=in1,
                            op=mybir.AluOpType.max,
                        )
                        tts.append(t)
                        if not last:
                            for nbs in range(OCTW):
                                nb = oct_ * OCTW + nbs
                                bk, jc = nb % 8, nb // 8
                                nc.tensor.matmul(
                                    obanks[bk][32 * jc : 32 * jc + 1, :],
                                    w2_sb[:, c : c + 1],
                                    t[:, ts(nbs, 512)],
                                    start=(c == 0),
                                    stop=False,
                                    tile_position=(0, 32 * jc),
                                )
                    if last:
                        # bank-major: finish a bank's 4 slivers, evacuate them
                        # (DVE+ACT alternating) while the next bank matmuls
                        for bk in range(8):
                            for jc in range(NOCT):
                                nb = jc * 8 + bk
                                oct_, nbs = nb // OCTW, nb % OCTW
                                nc.tensor.matmul(
                                    obanks[bk][32 * jc : 32 * jc + 1, :],
                                    w2_sb[:, c : c + 1],
                                    tts[oct_][:, ts(nbs, 512)],
                                    start=False,
                                    stop=True,
                                    tile_position=(0, 32 * jc),
                                )
                            for jc in range(NOCT):
                                nb = jc * 8 + bk
                                src = obanks[bk][32 * jc : 32 * jc + 1, :]
                                if bk < 4:
                                    dst = out_sbA[
                                        :, (jc * 4 + bk) * 512 : (jc * 4 + bk + 1) * 512
                                    ]
                                    nc.scalar.copy(out=dst, in_=src)
                                else:
                                    dst = out_sbB[
                                        :,
                                        (jc * 4 + bk - 4) * 512 : (jc * 4 + bk - 3)
                                        * 512,
                                    ]
                                    nc.vector.tensor_copy(out=dst, in_=src)

            # raw[nb*512 + m*4 + j] with nb = jc*8 + bk; A holds bk 0-3,
            # B holds bk 4-7
            out_v = out[:, :].rearrange("o (jc bk q) -> o jc bk q", jc=4, bk=8)
            nc.sync.dma_start(
                out=out_v[:, :, 0:4, :],
                in_=out_sbA[:, :].rearrange("o (jc bk q) -> o jc bk q", jc=4, bk=4),
            )
            nc.sync.dma_start(
                out=out_v[:, :, 4:8, :],
                in_=out_sbB[:, :].rearrange("o (jc bk q) -> o jc bk q", jc=4, bk=4),
            )

    if do_compile:
        nc.compile()
    return nc


_NC_CACHE = None


def _get_nc():
    global _NC_CACHE
    if _NC_CACHE is None:
        _NC_CACHE = _build_nc()
    return _NC_CACHE


def prepare_in_maps(X, Y, W1, b1, W2):
    X = np.asarray(X, dtype=np.float32)
    Y = np.asarray(Y, dtype=np.float32)
    W1 = np.asarray(W1, dtype=np.float32)
    b1 = np.asarray(b1, dtype=np.float32)
    W2 = np.asarray(W2, dtype=np.float32)

    bf = ml_dtypes.bfloat16
    # chunk-major (NCH*D, 128): rows c*D..(c+1)*D = W1{x,y}.T[:, c*128:(c+1)*128]
    w1xt = np.ascontiguousarray(
        W1[:, :D].T.reshape(D, NCH, 128).transpose(1, 0, 2).reshape(NCH * D, 128)
    ).astype(bf)
    w1yt = np.ascontiguousarray(
        W1[:, D:].T.reshape(D, NCH, 128).transpose(1, 0, 2).reshape(NCH * D, 128)
    ).astype(bf)
    b1cm = np.ascontiguousarray(b1.reshape(NCH, 128).T)       # (128, NCH) f32
    w2cm = np.ascontiguousarray(W2.reshape(NCH, 128).T).astype(bf)

    in_maps = []
    for b in range(B):
        in_maps.append(
            {
                "xt": np.ascontiguousarray(X[b].T).astype(bf),
                "yt": np.ascontiguousarray(Y[b].T).astype(bf),
                "w1xt": w1xt,
                "w1yt": w1yt,
                "b1c": b1cm,
                "w2c": w2cm,
            }
        )
    return in_maps


def postprocess(raw_outs, X, W1, b1, W2, b2):
    """raw[nb*512 + m*4 + j] = Sum_h W2[h]*u for out row 4nb+j, col m.
    Add gamma[b,n] = (W2@W1x)·X[b,n] + W2·b1, then b2."""
    X = np.asarray(X, dtype=np.float32)
    W1 = np.asarray(W1, dtype=np.float32)
    b1 = np.asarray(b1, dtype=np.float32)
    W2 = np.asarray(W2, dtype=np.float32)
    b2 = np.asarray(b2, dtype=np.float32)

    v = W2[0] @ W1[:, :D]                     # (D,)
    gconst = float(W2[0] @ b1)
    out = np.empty((B, NX, NY), dtype=np.float32)
    for b in range(B):
        r = raw_outs[b].reshape(NBLK, 128, 4)     # (nb, m, j)
        o = r.transpose(0, 2, 1).reshape(NX, NY)  # (4nb+j, m)
        gamma = X[b] @ v + gconst                 # (NX,)
        out[b] = o + gamma[:, None] + b2[0]
    return out


def kernel(X, Y, W1, b1, W2, b2):
    in_maps = prepare_in_maps(X, Y, W1, b1, W2)
    nc = _get_nc()
    res = run_bass_kernel_spmd(nc, in_maps, core_ids=list(range(NCORES)))
    raw = [res.results[b]["out"].reshape(-1) for b in range(B)]
    return postprocess(raw, X, W1, b1, W2, b2)


if __name__ == "__main__":
    rng = np.random.default_rng(0)
    ins = {
        "X": rng.standard_normal((B, NX, D), dtype=np.float32),
        "Y": rng.standard_normal((B, NY, D), dtype=np.float32),
        "W1": rng.standard_normal((HD, 2 * D), dtype=np.float32) * (2 * D) ** -0.5,
        "b1": rng.standard_normal((HD,), dtype=np.float32) * (2 * D) ** -0.5,
        "W2": rng.standard_normal((1, HD), dtype=np.float32) * HD**-0.5,
        "b2": rng.standard_normal((1,), dtype=np.float32) * HD**-0.5,
    }
    o = kernel(**ins)
    print("kernel out:", o.shape, o.dtype, float(np.abs(o).max()))
